# revision 14
# baseline (speedup 1.0000x reference)
"""GQA attention + RoPE + causal softmax + output projection on 8 TRN2 cores.

Sharding: tensor-parallel over heads. Core i owns q-heads [4i, 4i+4) and
kv-head i (GQA group size 4 == HQ/8, HK/8 = 1).

fp8 strategy (keeps rel err ~1%, well under the 2e-2 gate):
  - Q and K paths run PURE fp8e4 (x, wq, wk, and the rope'd Q^T/K^T all fp8):
    score errors are ~5% of |s| with |s| ~ 4e-3, so p = exp(s) moves by
    ~2e-4 absolute -- diluted to ~0.03% on the output by the softmax.
  - V path and the output projection carry first-order residual corrections:
    x = x8 + xr8, wv = wv8 + wvr8, attn = ao8 + aor8, wo = wo8 + wor8 (all
    fp8 pairs; the resid*resid cross term is dropped, ~0.4% second order).
  - All fp8 matmuls use MatmulPerfMode.DoubleRow (2 contraction subtiles per
    instruction at 0.5 cycles/output-column = 4x bf16 throughput). The
    HD=128 score contraction is split as [64 partitions x 2 subtiles].
  - P*V stays bf16 (p values cluster at 1.0; fp8 would quantize away the
    softmax signal).

Scale bookkeeping (powers of two, folded into existing constants):
  x8,w8 carry 2^7 each -> projection PSUM = 2^14 * true.
  cos2/sin2 carry 2^-8   -> Q^T/K^T fp8 = 2^6 * true; score PSUM = 2^12 * s.
  exp scale = 2^-12/sqrt(HD).  V stays scaled: vts = 2^14 * v.
  ones_mat = 16 = 2^(14-10)  -> ao = 2^10 * attn (good fp8 range).
  out PSUM = 2^(10+7) * true -> final ACT copy uses scale 2^-17.

Softmax denominator: DVE accumulates exp chunks into two bf16 accumulators
(even/odd chunks, so the serial add chain keeps up with the PE), then two
ones-matmuls fold the partition sum + broadcast + 2^4 scale in one step.

Collectives: two AllGathers over sequence halves (all 4 heads + resid rows
per half), issued after qb=1 and qb=3 of the qb-outer attention loop; the
output projection consumes half 0 while half 1 is still being gathered.
"""

import os

import numpy as np
import ml_dtypes

import concourse.bass as bass
import concourse.mybir as mybir
import concourse.tile as tile
from concourse import bacc
from concourse.bass_utils import run_bass_kernel_spmd

# Problem dims (hardcoded per contract)
B, S, D = 1, 2048, 4096
HQ, HK, HD = 32, 8, 128
NCORES = 8
HQL = HQ // NCORES          # 4 local q heads
SB = 512                    # seq block (matmul moving free dim)
NB = S // SB                # 4 seq blocks
NPAIR = D // 256            # 16 DoubleRow contraction pairs for D
SCALE = 1.0 / float(np.sqrt(HD))
EXP_SCALE = SCALE / 4096.0  # scores PSUM carries 2^12
S2 = S // 2                 # gather half width

F32 = mybir.dt.float32
BF16 = mybir.dt.bfloat16
FP8 = mybir.dt.float8e4
DR = mybir.MatmulPerfMode.DoubleRow
DEBUG_DUMPS = os.environ.get("BASSDBG", "") == "1"

# stream_shuffle mask: swap adjacent pairs within each 32-partition quadrant
SWAP_MASK = [(i ^ 1) for i in range(32)]


def _build_nc():
    nc = bacc.Bacc(
        "TRN2", target_bir_lowering=False, debug=False, num_devices=NCORES
    )

    io = {}
    io["x8"] = nc.dram_tensor("x8", [D, S], FP8, kind="ExternalInput")
    io["xr8"] = nc.dram_tensor("xr8", [D, S], FP8, kind="ExternalInput")
    io["wq8"] = nc.dram_tensor("wq8", [D, HQL * HD], FP8, kind="ExternalInput")
    io["wk8"] = nc.dram_tensor("wk8", [D, HD], FP8, kind="ExternalInput")
    io["wv8"] = nc.dram_tensor("wv8", [D, HD], FP8, kind="ExternalInput")
    io["wvr8"] = nc.dram_tensor("wvr8", [D, HD], FP8, kind="ExternalInput")
    io["wo8"] = nc.dram_tensor("wo8", [D, HQL * HD], FP8, kind="ExternalInput")
    io["wor8"] = nc.dram_tensor("wor8", [D, HQL * HD], FP8, kind="ExternalInput")
    io["cos2"] = nc.dram_tensor("cos2", [HD, S], BF16, kind="ExternalInput")
    io["sin2"] = nc.dram_tensor("sin2", [HD, S], BF16, kind="ExternalInput")
    io["maskt"] = nc.dram_tensor("maskt", [128, NB, SB], BF16, kind="ExternalInput")
    io["ident"] = nc.dram_tensor("ident", [128, 128], BF16, kind="ExternalInput")
    io["outT"] = nc.dram_tensor("outT", [HQL * HD, S], F32, kind="ExternalOutput")
    if DEBUG_DUMPS:
        io["dbg_qt"] = nc.dram_tensor(
            "dbg_qt", [128, NB, HQL, SB], FP8, kind="ExternalOutput"
        )
        io["dbg_kt"] = nc.dram_tensor(
            "dbg_kt", [128, NB, SB], FP8, kind="ExternalOutput"
        )
        io["dbg_vs"] = nc.dram_tensor(
            "dbg_vs", [128, NB, SB // 128, HD], BF16, kind="ExternalOutput"
        )
        io["dbg_loc"] = nc.dram_tensor(
            "dbg_loc", [2 * HQL * HD, S2], FP8, kind="ExternalOutput"
        )

    with tile.TileContext(nc) as tc:
        _body(tc, io)
    nc.compile()
    return nc


def _body(tc, io):
    nc = tc.nc
    from contextlib import ExitStack

    ctx = ExitStack()
    with ctx:
        consts = ctx.enter_context(tc.tile_pool(name="consts", bufs=1))
        qkv = ctx.enter_context(tc.tile_pool(name="qkv", bufs=1))
        dram = ctx.enter_context(tc.tile_pool(name="dram", bufs=1, space="DRAM"))

        cos2 = consts.tile([HD, S], BF16)
        sin2 = consts.tile([HD, S], BF16)
        ident = consts.tile([128, 128], BF16)
        maskt = consts.tile([128, NB, SB], BF16)
        ones_mat = consts.tile([128, 128], BF16)
        nc.vector.memset(ones_mat, 16.0)

        # persistent per-core tensors, split per s-block for fine-grained deps
        qt8f = [
            qkv.tile([128, HQL, SB], FP8, name=f"qt8f{sb}") for sb in range(NB)
        ]
        kt8f = [qkv.tile([128, SB], FP8, name=f"kt8f{sb}") for sb in range(NB)]
        # [64, 2, ...] split layouts for DoubleRow score matmuls
        qt8 = [
            qkv.tile([64, 2, HQL, SB], FP8, name=f"qt8_{sb}") for sb in range(NB)
        ]
        kt8 = [qkv.tile([64, 2, SB], FP8, name=f"kt8_{sb}") for sb in range(NB)]
        vs_sb = [
            qkv.tile([128, SB // 128, HD], BF16, name=f"vs{sb}") for sb in range(NB)
        ]

        # half-sequence bounce + gather buffers: rows 0-511 = ao8 (4 heads),
        # rows 512-1023 = aor residual
        attn_loc = [
            dram.tile([2 * HQL * HD, S2], FP8, name=f"attn_loc{g2}")
            for g2 in range(2)
        ]
        attn_g = [
            dram.tile(
                [NCORES * 2 * HQL * HD, S2],
                FP8,
                name=f"attn_g{g2}",
                addr_space="Shared",
            )
            for g2 in range(2)
        ]

        # ================= Stage A: projections + RoPE =================
        with ctx_pools(tc) as (wpool, xpool, rpool, psA):
            # warm-up DoubleRow (discarded): the first dual-fp8 ldweights in a
            # program mis-executes (partial-NaN psum); absorb it on zeros.
            dmy_l = wpool.tile([128, 2, 128], FP8, name="dmy_l")
            dmy_r = wpool.tile([128, 2, 8], FP8, name="dmy_r")
            nc.vector.memset(dmy_l, 0.0)
            nc.vector.memset(dmy_r, 0.0)
            ps_warm = psA.tile([128, 8], F32, name="ps_warm", tag="psvt")
            nc.tensor.matmul(
                ps_warm, lhsT=dmy_l, rhs=dmy_r, start=True, stop=True, perf_mode=DR
            )

            wq_sb = wpool.tile([128, 2 * NPAIR, HQL * HD], FP8)
            wk_sb = wpool.tile([128, 2 * NPAIR, HD], FP8)
            wv_sb = wpool.tile([128, 2 * NPAIR, HD], FP8)
            wvr_sb = wpool.tile([128, 2 * NPAIR, HD], FP8)
            # first pair as fine slices so the PE can start ASAP
            for t in range(HQL):
                nc.gpsimd.dma_start(
                    out=wq_sb[:, 0:2, t * 128 : (t + 1) * 128],
                    in_=io["wq8"][0:256, t * 128 : (t + 1) * 128].rearrange(
                        "(c p) n -> p c n", p=128
                    ),
                )
            nc.gpsimd.dma_start(
                out=wk_sb[:, 0:2, :],
                in_=io["wk8"][0:256, :].rearrange("(c p) n -> p c n", p=128),
            )
            nc.gpsimd.dma_start(
                out=wv_sb[:, 0:2, :],
                in_=io["wv8"][0:256, :].rearrange("(c p) n -> p c n", p=128),
            )
            nc.gpsimd.dma_start(
                out=wvr_sb[:, 0:2, :],
                in_=io["wvr8"][0:256, :].rearrange("(c p) n -> p c n", p=128),
            )
            # pair 1 (the c4 bulk loop below starts at chunk 4)
            for w_sb, nm in (
                (wq_sb, "wq8"),
                (wk_sb, "wk8"),
                (wv_sb, "wv8"),
                (wvr_sb, "wvr8"),
            ):
                nc.gpsimd.dma_start(
                    out=w_sb[:, 2:4, :],
                    in_=io[nm][256:512, :].rearrange("(c p) n -> p c n", p=128),
                )
            # consts next on the queue: rope for s-block 0 needs cos2/sin2
            # before the bulk weight transfers finish
            nc.gpsimd.dma_start(out=ident, in_=io["ident"][:, :])
            nc.gpsimd.dma_start(out=cos2, in_=io["cos2"][:, :])
            nc.gpsimd.dma_start(out=sin2, in_=io["sin2"][:, :])
            nc.gpsimd.dma_start(out=maskt, in_=io["maskt"][:, :, :])
            for c4 in range(1, NPAIR // 2):
                sl = slice(c4 * 512, (c4 + 1) * 512)
                for w_sb, nm in (
                    (wq_sb, "wq8"),
                    (wk_sb, "wk8"),
                    (wv_sb, "wv8"),
                    (wvr_sb, "wvr8"),
                ):
                    nc.gpsimd.dma_start(
                        out=w_sb[:, c4 * 4 : c4 * 4 + 4, :],
                        in_=io[nm][sl, :].rearrange("(c p) n -> p c n", p=128),
                    )

            for sb in range(NB):
                ssl = slice(sb * SB, (sb + 1) * SB)
                ps_q = [
                    psA.tile(
                        [128, SB],
                        F32,
                        name=f"psq{t}_{sb}",
                        tag=f"psq{t}",
                        bufs=2 if t == 0 else 1,
                    )
                    for t in range(HQL)
                ]
                ps_k = psA.tile([128, SB], F32, tag="psk")
                ps_v = psA.tile([128, SB], F32, tag="psv")
                for c in range(NPAIR):
                    # [128, 4, SB]: subtiles 0:2 = x8 pair, 2:4 = xr8 pair
                    xt = xpool.tile([128, 4, SB], FP8, tag="xt")
                    rsl = slice(c * 256, (c + 1) * 256)
                    nc.sync.dma_start(
                        out=xt[:, 0:2, :],
                        in_=io["x8"][rsl, ssl].rearrange("(c p) n -> p c n", p=128),
                    )
                    nc.scalar.dma_start(
                        out=xt[:, 2:4, :],
                        in_=io["xr8"][rsl, ssl].rearrange("(c p) n -> p c n", p=128),
                    )
                    first, last = c == 0, c == NPAIR - 1
                    x8p = xt[:, 0:2, :]
                    wsl = slice(2 * c, 2 * c + 2)
                    for t in range(HQL):
                        nc.tensor.matmul(
                            ps_q[t],
                            lhsT=wq_sb[:, wsl, t * 128 : (t + 1) * 128],
                            rhs=x8p,
                            start=first,
                            stop=last,
                            perf_mode=DR,
                        )
                    nc.tensor.matmul(
                        ps_k, lhsT=wk_sb[:, wsl, :], rhs=x8p,
                        start=first, stop=last, perf_mode=DR,
                    )
                    nc.tensor.matmul(
                        ps_v, lhsT=wv_sb[:, wsl, :], rhs=x8p,
                        start=first, stop=False, perf_mode=DR,
                    )
                    nc.tensor.matmul(
                        ps_v, lhsT=wvr_sb[:, wsl, :], rhs=x8p,
                        start=False, stop=False, perf_mode=DR,
                    )
                    nc.tensor.matmul(
                        ps_v, lhsT=wv_sb[:, wsl, :], rhs=xt[:, 2:4, :],
                        start=False, stop=last, perf_mode=DR,
                    )

                # V^T -> V (PE transpose per 128-col chunk)
                vts = rpool.tile([128, SB], BF16, name=f"vts{sb}", tag="vts")
                nc.scalar.copy(vts, ps_v)
                for u in range(SB // 128):
                    ps_vt = psA.tile([128, 128], BF16, name=f"psvt{sb}_{u}", tag="psvt")
                    nc.tensor.transpose(
                        ps_vt, vts[:, u * 128 : (u + 1) * 128], ident
                    )
                    nc.vector.tensor_copy(vs_sb[sb][:, u, :], ps_vt)

                # RoPE -> fp8: rot(q) = q*cos2 + pairswap(q)*sin2, all bf16
                # muls, fp8 destination. cos2/sin2 carry 2^-8.
                def rope(ps, dst, idx):
                    qc = rpool.tile([128, SB], BF16, name=f"qc{idx}", tag="qc")
                    nc.scalar.copy(qc, ps)
                    sw = rpool.tile([128, SB], BF16, name=f"sw{idx}", tag="sw")
                    nc.vector.stream_shuffle(sw, qc, SWAP_MASK)
                    t1 = rpool.tile([128, SB], BF16, name=f"t1{idx}", tag="t1")
                    nc.vector.tensor_mul(t1, qc, cos2[:, ssl])
                    t2 = rpool.tile([128, SB], BF16, name=f"t2{idx}", tag="t2")
                    nc.vector.tensor_mul(t2, sw, sin2[:, ssl])
                    nc.vector.tensor_add(dst, t1, t2)

                for t in range(HQL):
                    rope(ps_q[t], qt8f[sb][:, t, :], f"q{sb}_{t}")
                rope(ps_k, kt8f[sb], f"k{sb}")

                # split [128, .] -> [64, 2, .] for DoubleRow score matmuls
                for t in range(HQL):
                    nc.gpsimd.dma_start(out=qt8[sb][:, 0, t, :], in_=qt8f[sb][0:64, t, :])
                    nc.gpsimd.dma_start(out=qt8[sb][:, 1, t, :], in_=qt8f[sb][64:128, t, :])
                nc.gpsimd.dma_start(out=kt8[sb][:, 0, :], in_=kt8f[sb][0:64, :])
                nc.gpsimd.dma_start(out=kt8[sb][:, 1, :], in_=kt8f[sb][64:128, :])
                if DEBUG_DUMPS:
                    nc.gpsimd.dma_start(out=io["dbg_qt"][:, sb, :, :], in_=qt8f[sb])
                    nc.gpsimd.dma_start(out=io["dbg_kt"][:, sb, :], in_=kt8f[sb])
                    nc.gpsimd.dma_start(out=io["dbg_vs"][:, sb, :, :], in_=vs_sb[sb])

        # wo loads fill DMA idle time during stage B
        wo_pool = ctx.enter_context(tc.tile_pool(name="wo_pool", bufs=1))
        wo_sb = wo_pool.tile([128, 2 * NPAIR, HQL * HD], FP8)
        wor_sb = wo_pool.tile([128, 2 * NPAIR, HQL * HD], FP8)
        for c4 in range(NPAIR // 2):
            sl = slice(c4 * 512, (c4 + 1) * 512)
            nc.gpsimd.dma_start(
                out=wo_sb[:, c4 * 4 : c4 * 4 + 4, :],
                in_=io["wo8"][sl, :].rearrange("(c p) n -> p c n", p=128),
            )
            nc.gpsimd.dma_start(
                out=wor_sb[:, c4 * 4 : c4 * 4 + 4, :],
                in_=io["wor8"][sl, :].rearrange("(c p) n -> p c n", p=128),
            )

        apool = ctx.enter_context(tc.tile_pool(name="apool", bufs=10))
        opool = ctx.enter_context(tc.tile_pool(name="opool", bufs=4))

        # ================= Stage B: attention (qb outer) =================
        # Chunk pipeline: PV of chunk kc-2 is emitted after exp of chunk kc so
        # the PE never waits on ACT latency. Denominator: DVE accumulates exp
        # chunks into 2 bf16 accumulators; two ones-matmuls do partition-sum +
        # broadcast + 2^4 scale.
        with ctx_pools_b(tc) as (ppool, spool, psB):
            for qb in range(NB):
                for h in range(HQL):
                    qsl = slice(qb * SB, (qb + 1) * SB)
                    nkc = (qb + 1) * (SB // 128)
                    ps_o = psB.tile([128, SB], F32, name=f"pso{h}_{qb}", tag="pso")
                    acc = [
                        spool.tile(
                            [128, SB], BF16, name=f"acc{e}_{h}_{qb}", tag=f"acc{e}"
                        )
                        for e in range(2)
                    ]
                    acc_init = [False, False]
                    pts = {}

                    def consume(kc, h=h, qb=qb, ps_o=ps_o, pts=pts):
                        first, last = kc == 0, kc == nkc - 1
                        pt = pts.pop(kc)
                        nc.tensor.matmul(
                            ps_o,
                            lhsT=vs_sb[kc // 4][:, kc % 4, :],
                            rhs=pt,
                            start=first,
                            stop=last,
                        )

                    for kc in range(nkc):
                        ps_s = psB.tile(
                            [128, SB], F32, name=f"pss{h}_{qb}_{kc}", tag="pss", bufs=4
                        )
                        nc.tensor.matmul(
                            ps_s,
                            lhsT=kt8[kc // 4][:, :, (kc % 4) * 128 : (kc % 4 + 1) * 128],
                            rhs=qt8[qb][:, :, h, :],
                            start=True,
                            stop=True,
                            perf_mode=DR,
                        )
                        pt = ppool.tile(
                            [128, SB], BF16, name=f"pt{h}_{qb}_{kc}", tag="pt"
                        )
                        nc.scalar.activation(
                            pt, ps_s, mybir.ActivationFunctionType.Exp,
                            scale=EXP_SCALE,
                        )
                        td = kc - qb * 4
                        if td >= 0:
                            ptm = ppool.tile(
                                [128, SB], BF16, name=f"ptm{h}_{qb}_{kc}", tag="ptm",
                                bufs=4,
                            )
                            nc.vector.tensor_mul(ptm, pt, maskt[:, td, :])
                            pt = ptm
                        pts[kc] = pt
                        e = kc % 2
                        if acc_init[e]:
                            nc.vector.tensor_add(acc[e], acc[e], pt)
                        else:
                            nc.vector.tensor_copy(acc[e], pt)
                            acc_init[e] = True
                        if kc >= 2:
                            consume(kc - 2)
                    if nkc >= 2:
                        consume(nkc - 2)
                    consume(nkc - 1)

                    ps_n = psB.tile([128, SB], F32, name=f"psn{h}_{qb}", tag="psn")
                    nc.tensor.matmul(
                        ps_n, lhsT=ones_mat, rhs=acc[0], start=True,
                        stop=not acc_init[1],
                    )
                    if acc_init[1]:
                        nc.tensor.matmul(
                            ps_n, lhsT=ones_mat, rhs=acc[1], start=False, stop=True
                        )
                    rb = spool.tile([128, SB], F32, name=f"rb{h}_{qb}", tag="rb")
                    nc.vector.reciprocal_approx_fast(rb, ps_n)
                    aof = spool.tile(
                        [128, SB], F32, name=f"aof{h}_{qb}", tag="aof", bufs=3
                    )
                    nc.vector.tensor_mul(aof, ps_o, rb)
                    ao8 = spool.tile(
                        [128, SB], FP8, name=f"ao8{h}_{qb}", tag="ao8", bufs=4
                    )
                    nc.gpsimd.tensor_copy(ao8, aof)
                    aor = spool.tile(
                        [128, SB], FP8, name=f"aor{h}_{qb}", tag="aor", bufs=4
                    )
                    nc.gpsimd.tensor_sub(aor, aof, ao8)
                    g2 = qb // 2
                    csl = slice((qb % 2) * SB, (qb % 2) * SB + SB)
                    nc.gpsimd.dma_start(
                        out=attn_loc[g2][h * 128 : (h + 1) * 128, csl], in_=ao8
                    )
                    nc.gpsimd.dma_start(
                        out=attn_loc[g2][512 + h * 128 : 512 + (h + 1) * 128, csl],
                        in_=aor,
                    )
                if DEBUG_DUMPS and qb == 1:
                    nc.gpsimd.dma_start(
                        out=io["dbg_loc"][:, :], in_=attn_loc[0][:, :]
                    )
                if qb % 2 == 1:
                    g2 = qb // 2
                    nc.gpsimd.collective_compute(
                        "AllGather",
                        mybir.AluOpType.bypass,
                        replica_groups=[list(range(NCORES))],
                        ins=[attn_loc[g2].opt()],
                        outs=[attn_g[g2].opt()],
                    )

        # ======== Stage D: out = (ao8+aor) @ (wo8+wor8), column shard ========
        # Global contraction chunk for (core i, head j) = 4i + j; main rows of
        # gather g2 at i*1024 + j*128, resid rows at i*1024 + 512 + j*128.
        # Three DoubleRow streams per pair p (chunks 2p, 2p+1):
        #   ao8*wo8, ao8*wor8 (same moving tile), aor*wo8.
        with tc.tile_pool(name="psD", bufs=2, space="PSUM") as psD:
            for g in range(NB):
                g2, gh = g // 2, g % 2
                gsl = slice(gh * SB, (gh + 1) * SB)
                osl = slice(g * SB, (g + 1) * SB)
                ps_d = [
                    psD.tile([128, SB], F32, name=f"psd{g}_{n}", tag=f"psd{n}")
                    for n in range(HQL)
                ]
                for p in range(NPAIR):
                    i, jp = p // 2, p % 2
                    am = apool.tile([128, 2, SB], FP8, tag="am")
                    ar = apool.tile([128, 2, SB], FP8, tag="ar")
                    row_m = i * 1024 + jp * 256
                    row_r = i * 1024 + 512 + jp * 256
                    nc.sync.dma_start(
                        out=am,
                        in_=attn_g[g2][row_m : row_m + 256, gsl].rearrange(
                            "(c p) n -> p c n", p=128
                        ),
                    )
                    nc.scalar.dma_start(
                        out=ar,
                        in_=attn_g[g2][row_r : row_r + 256, gsl].rearrange(
                            "(c p) n -> p c n", p=128
                        ),
                    )
                    first, last = p == 0, p == NPAIR - 1
                    wsl = slice(2 * p, 2 * p + 2)
                    for n in range(HQL):
                        nsl = slice(n * 128, (n + 1) * 128)
                        nc.tensor.matmul(
                            ps_d[n], lhsT=wo_sb[:, wsl, nsl], rhs=am,
                            start=first, stop=False, perf_mode=DR,
                        )
                        nc.tensor.matmul(
                            ps_d[n], lhsT=wor_sb[:, wsl, nsl], rhs=am,
                            start=False, stop=False, perf_mode=DR,
                        )
                        nc.tensor.matmul(
                            ps_d[n], lhsT=wo_sb[:, wsl, nsl], rhs=ar,
                            start=False, stop=last, perf_mode=DR,
                        )
                for n in range(HQL):
                    ot = opool.tile([128, SB], F32, name=f"ot{g}_{n}", tag="ot")
                    nc.scalar.activation(
                        ot, ps_d[n], mybir.ActivationFunctionType.Copy,
                        scale=1.0 / (1 << 17),
                    )
                    nc.scalar.dma_start(
                        out=io["outT"][n * 128 : (n + 1) * 128, osl], in_=ot
                    )


from contextlib import contextmanager


@contextmanager
def ctx_pools(tc):
    with (
        tc.tile_pool(name="wpool", bufs=1) as wpool,
        tc.tile_pool(name="xpool", bufs=8) as xpool,
        tc.tile_pool(name="rpool", bufs=3) as rpool,
        tc.tile_pool(name="psA", bufs=1, space="PSUM") as psA,
    ):
        yield wpool, xpool, rpool, psA


@contextmanager
def ctx_pools_b(tc):
    with (
        tc.tile_pool(name="ppool", bufs=8) as ppool,
        tc.tile_pool(name="spool", bufs=2) as spool,
        tc.tile_pool(name="psB", bufs=2, space="PSUM") as psB,
    ):
        yield ppool, spool, psB


_NC_CACHE = None


def _get_nc():
    global _NC_CACHE
    if _NC_CACHE is None:
        _NC_CACHE = _build_nc()
    return _NC_CACHE


def _prep_in_maps(x, freqs_cos, freqs_sin, wq, wk, wv, wo):
    bf = ml_dtypes.bfloat16
    f8 = ml_dtypes.float8_e4m3
    S7 = 128.0

    x = np.asarray(x, np.float32).reshape(S, D)
    xT = np.ascontiguousarray(x.T) * S7
    x8 = xT.astype(f8)
    xr8 = (xT - x8.astype(np.float32)).astype(f8)

    cos = np.asarray(freqs_cos, np.float32)  # [S, HD/2]
    sin = np.asarray(freqs_sin, np.float32)
    cos2 = np.repeat(cos.T, 2, axis=0)  # [HD, S]
    sin_t = sin.T
    sin2 = np.empty((HD, S), np.float32)
    sin2[0::2] = -sin_t
    sin2[1::2] = sin_t
    rs = 1.0 / 256.0  # 2^-8: descale 2^-14, rescale 2^6 for fp8 q/k
    cos2 = cos2 * rs
    sin2 = sin2 * rs

    p = np.arange(128)[:, None, None]
    t = np.arange(NB)[None, :, None]
    c = np.arange(SB)[None, None, :]
    maskt = (128 * t + p <= c).astype(bf)
    ident = np.eye(128, dtype=bf)

    wq = np.asarray(wq, np.float32) * S7
    wk = np.asarray(wk, np.float32) * S7
    wv = np.asarray(wv, np.float32) * S7
    wo = np.asarray(wo, np.float32) * S7
    in_maps = []
    for i in range(NCORES):
        wq_i = np.ascontiguousarray(wq[:, i * HQL * HD : (i + 1) * HQL * HD])
        wk_i = np.ascontiguousarray(wk[:, i * HD : (i + 1) * HD])
        wv_i = np.ascontiguousarray(wv[:, i * HD : (i + 1) * HD])
        wo_i = np.ascontiguousarray(wo[:, i * HQL * HD : (i + 1) * HQL * HD])
        wv8 = wv_i.astype(f8)
        wvr8 = (wv_i - wv8.astype(np.float32)).astype(f8)
        wo8 = wo_i.astype(f8)
        wor8 = (wo_i - wo8.astype(np.float32)).astype(f8)
        in_maps.append(
            {
                "x8": x8,
                "xr8": xr8,
                "cos2": cos2.astype(bf),
                "sin2": sin2.astype(bf),
                "maskt": maskt,
                "ident": ident,
                "wq8": wq_i.astype(f8),
                "wk8": wk_i.astype(f8),
                "wv8": wv8,
                "wvr8": wvr8,
                "wo8": wo8,
                "wor8": wor8,
            }
        )
    return in_maps


def _install_trace_shims():
    """The container's antenv lacks axon_hooks; replicate trn_boot's ctypes
    NTFF hook so run_bass_kernel_spmd(trace=True) works. Also stub out the
    fish-bucket artifact upload (no bucket access here)."""
    import sys
    import types
    import ctypes
    import contextlib

    if "antenv.axon_hooks" not in sys.modules:
        mod = types.ModuleType("antenv.axon_hooks")
        mod._hook = None

        def set_axon_ntff_profile_hook(h):
            mod._hook = h

        def get_axon_ntff_profile_hook():
            return mod._hook

        mod.set_axon_ntff_profile_hook = set_axon_ntff_profile_hook
        mod.get_axon_ntff_profile_hook = get_axon_ntff_profile_hook
        sys.modules["antenv.axon_hooks"] = mod

        so_path = "/opt/axon/libaxon_pjrt.so"
        lib = ctypes.CDLL(so_path)
        if hasattr(lib, "axon_start_nrt_profile"):
            lib.axon_start_nrt_profile.argtypes = [
                ctypes.POINTER(ctypes.c_int64),
                ctypes.c_size_t,
            ]
            lib.axon_start_nrt_profile.restype = ctypes.c_int64
            lib.axon_stop_nrt_profile.argtypes = [ctypes.c_char_p]
            lib.axon_stop_nrt_profile.restype = ctypes.c_int64

            @contextlib.contextmanager
            def _hook(output_dir, device_ids):
                import jax

                jax.devices()
                if device_ids:
                    ids = (ctypes.c_int64 * len(device_ids))(*device_ids)
                    rc = lib.axon_start_nrt_profile(ids, len(device_ids))
                else:
                    rc = lib.axon_start_nrt_profile(None, 0)
                if rc != 0:
                    raise RuntimeError(f"axon_start_nrt_profile rc={rc}")
                try:
                    yield
                finally:
                    n = lib.axon_stop_nrt_profile(str(output_dir).encode())
                    if n <= 0:
                        print(f"WARNING: axon_stop_nrt_profile rc={n}")

            set_axon_ntff_profile_hook(_hook)

    import concourse.bass_utils as bu

    bu.upload_artifacts = lambda tmpdir: "local://" + str(tmpdir)


def run(inputs, trace=False, **kw):
    nc = _get_nc()
    if trace:
        _install_trace_shims()
    in_maps = _prep_in_maps(**inputs)
    res = run_bass_kernel_spmd(nc, in_maps, list(range(NCORES)), trace=trace, **kw)
    out = np.concatenate(
        [res.results[i]["outT"].T for i in range(NCORES)], axis=1
    )
    return out.reshape(B, S, D).astype(np.float32), res


def kernel(x, freqs_cos, freqs_sin, wq, wk, wv, wo):
    out, _ = run(
        dict(
            x=x,
            freqs_cos=freqs_cos,
            freqs_sin=freqs_sin,
            wq=wq,
            wk=wk,
            wv=wv,
            wo=wo,
        )
    )
    return out
